# revision 7
# baseline (speedup 1.0000x reference)
"""TRN2 Bass kernel for the ESN (echo-state-network) recurrence:

    U   = inputs @ W_in + b_in                              # [B, T, N]
    x0  = 0.5 * tanh(U[:, 0])
    x_t = 0.5*x_{t-1} + 0.5*tanh(U[:, t] + x_{t-1} @ W_res + b_res)
    X   = stack([x0 ... x_{T-1}], 1)                        # [B, T, N]

Sharding: TIME-parallel with fading-memory warmup, two interleaved
chunks per core.  The ESN map is strongly contracting (leak 0.5,
spectral radius 0.9): a chunk's initial state is reconstructed by
running L warmup steps from x=0, with init error decaying below the
arithmetic noise floor well before L=12 (measured).

The sequence is split into 16 chunks of 32 output steps; core c runs
chunks 2c and 2c+1, interleaved step-by-step (A0 B0 A1 B1 ...).  The
two recurrences are independent, so while chunk A's post-matmul chain
(tanh -> blend -> transpose -> fp8 cast) completes, the PE runs chunk
B's full step: the sequential-dependence latency is fully hidden.
Same program on every core (SPMD); only input slices differ.  Chunk 0
of core 0 starts at t=0, where the program's step-0 formula is exactly
the reference's x0.

Per-step design ("x-stationary" matmul orientation, fp8 DoubleRow):
  - z[b, n] accumulates in PSUM [128, 1024] as lhsT.T @ rhs with the
    *state* stationary (xT8 fp8 tiles, 4 DoubleRow loads) and
    16*W_res as the fp8 moving operand: 8 DoubleRow matmuls per step
    stream the full N x N recurrent weight in ~1us.
  - The input projection + bias (scaled by 16 to match) is a K=65
    fp16 chunk accumulated into the same PSUM; tanh applies scale=1/16.
  - tanh on ScalarE (PSUM -> fp16), leak blend on VectorE, state
    transpose via the DMA Xbar engine (dma_start_transpose, off all
    compute engines, one [128,512]->[128,4,128] op per half), fp16->fp8
    cast on VectorE, x/2 bookkeeping on GpSimd.
  - Output written per step as [B, chunk, s, N] fp16; the host slices
    each chunk's valid 32-step window and concatenates.
"""

import sys

sys.path.insert(0, "/opt/trn_rl_repo")

from contextlib import ExitStack

import numpy as np

try:  # persistent jit cache so repeated runs skip long compiles
    import jax

    jax.config.update("jax_compilation_cache_dir", "/var/tmp/jax_comp_cache")
    jax.config.update("jax_persistent_cache_min_compile_time_secs", 0.0)
    jax.config.update("jax_persistent_cache_min_entry_size_bytes", 0)
except Exception:
    pass

import concourse.bass as bass
import concourse.tile as tile
from concourse import bacc, mybir
from concourse.bass_utils import run_bass_kernel_spmd

F32 = mybir.dt.float32
F16 = mybir.dt.float16
F8 = mybir.dt.float8e4

N_CORES = 8
B = 128  # full batch on every core
T = 512
D = 64
N = 1024
NC = 8  # N chunks of 128
P = 128
HN = 512  # half of N (one PSUM bank of fp32)
L = 12  # warmup steps per chunk
C2 = 32  # output steps per chunk
S2 = C2 + L  # program steps per chunk
NCHUNK = T // C2  # 16 chunks; 2 per core, interleaved
WS = 16.0  # fp8 weight scale (dodges e4m3 denormals; tanh rescales)
TANH = mybir.ActivationFunctionType.Tanh
ALU = mybir.AluOpType
DR = mybir.MatmulPerfMode.DoubleRow


def build_kernel():
    nc = bacc.Bacc(None, target_bir_lowering=False)
    # host-pretransposed inputs: inputs_t[d, b, j, s] = inputs[b, t0(j)+s, d]
    inputs = nc.dram_tensor("inputs_t", [D, B, 2, S2], F32, kind="ExternalInput")
    W_in = nc.dram_tensor("W_in", [D, N], F32, kind="ExternalInput")
    b_in = nc.dram_tensor("b_in", [N], F32, kind="ExternalInput")
    W_res = nc.dram_tensor("W_res", [N, N], F32, kind="ExternalInput")
    b_res = nc.dram_tensor("b_res", [N], F32, kind="ExternalInput")
    Xs = nc.dram_tensor("Xs", [B, 2, S2, N], F16, kind="ExternalOutput")

    K = D + 1  # input dims + ones row (bias via wi row 64)

    with tile.TileContext(nc) as tc, ExitStack() as ctx:
        consts = ctx.enter_context(tc.tile_pool(name="consts", bufs=1))
        stage = ctx.enter_context(tc.tile_pool(name="stage", bufs=1))
        state = ctx.enter_context(tc.tile_pool(name="state", bufs=2))
        zpool = ctx.enter_context(
            tc.tile_pool(name="zpool", bufs=2, space=bass.MemorySpace.PSUM)
        )

        # ---- W_res: wres8[p, k, n] = e4m3(16 * W_res[k*128+p, n]) ----
        wst = stage.tile([P, NC * N], F32, tag="stage", name="wst")
        nc.gpsimd.dma_start(
            out=wst.rearrange("p (k n) -> p k n", n=N),
            in_=W_res[:].rearrange("(k p) n -> p k n", p=P),
        )
        wres8 = consts.tile([P, NC, N], F8, tag="wres8")
        nc.vector.tensor_scalar_mul(wres8.rearrange("p k n -> p (k n)"), wst, WS)

        # ---- wi tiles [65, N] (x16): rows 0..63 = 16*W_in; row 64 = 16*bias ----
        wi32 = consts.tile([K, N], F32, tag="wi32")
        wi032 = consts.tile([K, N], F32, tag="wi032")
        nc.gpsimd.dma_start(out=wi32[0:D], in_=W_in[:])
        nc.gpsimd.dma_start(out=wi032[0:D], in_=W_in[:])
        nc.gpsimd.dma_start(
            out=wi032[D : D + 1], in_=b_in[:].rearrange("(z n) -> z n", z=1)
        )
        bres_row = consts.tile([K, N], F32, tag="bres")
        nc.gpsimd.dma_start(
            out=bres_row[D : D + 1], in_=b_res[:].rearrange("(z n) -> z n", z=1)
        )
        nc.vector.tensor_tensor(
            out=wi32[D : D + 1],
            in0=wi032[D : D + 1],
            in1=bres_row[D : D + 1],
            op=ALU.add,
        )
        wi = consts.tile([K, N], F16, tag="wi")
        wi0 = consts.tile([K, N], F16, tag="wi0")
        nc.vector.tensor_scalar_mul(wi, wi32, WS)
        nc.vector.tensor_scalar_mul(wi0, wi032, WS)

        # ---- inputs: inp[d, b, j, s] fp16, row 64 = ones ----
        ist = stage.tile([D, B * 2 * S2], F32, tag="ist")
        nc.sync.dma_start(out=ist, in_=inputs[:].rearrange("d b j s -> d (b j s)"))
        inp = consts.tile([K, B, 2, S2], F16, tag="inp")
        nc.vector.tensor_copy(out=inp[0:D].rearrange("d b j s -> d (b j s)"), in_=ist)
        nc.vector.memset(inp[D : D + 1].rearrange("d b j s -> d (b j s)"), 1.0)

        xs_view = Xs[:]  # [B, 2, S2, N]
        hs0, hs1 = slice(0, HN), slice(HN, N)

        zt = {}

        def inp_mm(j, s):
            z = zpool.tile([P, N], F32, tag=f"z{j}", name=f"z{j}_{s}")
            zt[(j, s)] = z
            wi_use = wi0 if s == 0 else wi
            for h in range(2):
                nc.tensor.matmul(
                    z[:, h * HN : (h + 1) * HN],
                    inp[:, :, j, s],
                    wi_use[:, h * HN : (h + 1) * HN],
                    start=True,
                    stop=(s == 0),
                    skip_group_check=True,
                )

        # per-chunk rotating state
        xh_prev = [None, None]
        xT8_prev = [None, None]

        inp_mm(0, 0)
        inp_mm(1, 0)
        for i in range(2 * S2):
            j, s = i % 2, i // 2
            last = s == S2 - 1
            z = zt.pop((j, s))
            if s + 1 < S2:
                inp_mm(j, s + 1)
            if s > 0:
                xT8 = xT8_prev[j]
                for c in range(4):
                    for h in range(2):
                        nc.tensor.matmul(
                            z[:, h * HN : (h + 1) * HN],
                            xT8[:, 2 * c : 2 * c + 2, :],
                            wres8[:, 2 * c : 2 * c + 2, h * HN : (h + 1) * HN],
                            start=False,
                            stop=(c == 3),
                            perf_mode=DR,
                            skip_group_check=True,
                        )
            th = state.tile([P, N], F16, tag=f"th{j}", name=f"th{j}_{s}")
            xn = state.tile([P, N], F16, tag=f"xn{j}", name=f"xn{j}_{s}")
            for hs in (hs0, hs1):
                nc.scalar.activation(
                    out=th[:, hs], in_=z[:, hs], func=TANH, scale=1.0 / WS
                )
                if s == 0:
                    nc.vector.tensor_scalar_mul(xn[:, hs], th[:, hs], 0.5)
                else:
                    nc.vector.scalar_tensor_tensor(
                        out=xn[:, hs], in0=th[:, hs], scalar=0.5,
                        in1=xh_prev[j][:, hs], op0=ALU.mult, op1=ALU.add,
                    )
            if not last:
                # state transpose on the DMA Xbar (off compute engines),
                # then fp16 -> fp8 cast for next step's stationary operand
                xT16 = state.tile([P, NC, P], F16, tag=f"xT16{j}", name=f"xT16{j}_{s}")
                xT8 = state.tile([P, NC, P], F8, tag=f"xT8{j}", name=f"xT8{j}_{s}")
                for h, hs in ((0, hs0), (1, hs1)):
                    nc.sync.dma_start_transpose(
                        out=xT16[:, 4 * h : 4 * h + 4, :], in_=xn[:, hs]
                    )
                    nc.vector.tensor_copy(
                        out=xT8[:, 4 * h : 4 * h + 4, :].rearrange("p c b -> p (c b)"),
                        in_=xT16[:, 4 * h : 4 * h + 4, :].rearrange("p c b -> p (c b)"),
                    )
                xh = state.tile([P, N], F16, tag=f"xh{j}", name=f"xh{j}_{s}")
                nc.gpsimd.tensor_scalar_mul(xh[:, hs0], xn[:, hs0], 0.5)
                nc.gpsimd.tensor_scalar_mul(xh[:, hs1], xn[:, hs1], 0.5)
            else:
                xT8 = xh = None
            nc.sync.dma_start(out=xs_view[:, j, s, :], in_=xn)
            xh_prev[j], xT8_prev[j] = xh, xT8

    nc.compile()
    return nc


_NC_CACHE = {}


def _get_nc():
    if "nc" not in _NC_CACHE:
        _NC_CACHE["nc"] = build_kernel()
    return _NC_CACHE["nc"]


def _chunk_start(g):
    return 0 if g == 0 else C2 * g - L


def run_sharded(inputs, W_in, b_in, W_res, b_res, trace=False):
    """Run the SPMD kernel on 8 cores; returns (X_full, BassKernelResults)."""
    b_total, t_steps, _ = inputs.shape
    assert b_total == B and t_steps == T
    nc = _get_nc()
    shared = {
        "W_in": np.ascontiguousarray(W_in, np.float32),
        "b_in": np.ascontiguousarray(b_in, np.float32),
        "W_res": np.ascontiguousarray(W_res, np.float32),
        "b_res": np.ascontiguousarray(b_res, np.float32),
    }
    inputs = np.asarray(inputs, np.float32)
    in_maps = []
    for c in range(N_CORES):
        sl = np.empty((D, B, 2, S2), np.float32)
        for j in range(2):
            t0 = _chunk_start(2 * c + j)
            sl[:, :, j, :] = inputs[:, t0 : t0 + S2, :].transpose(2, 0, 1)
        in_maps.append({"inputs_t": np.ascontiguousarray(sl), **shared})
    res = run_bass_kernel_spmd(nc, in_maps, core_ids=list(range(N_CORES)), trace=trace)
    X = np.empty((B, T, N), np.float32)
    for c, r in enumerate(res.results):
        for j in range(2):
            g = 2 * c + j
            lo = 0 if g == 0 else L
            X[:, C2 * g : C2 * (g + 1), :] = r["Xs"][:, j, lo : lo + C2, :].astype(
                np.float32
            )
    return X, res


def kernel(**inputs):
    X, _ = run_sharded(
        inputs["inputs"],
        inputs["W_in"],
        inputs["b_in"],
        inputs["W_res"],
        inputs["b_res"],
    )
    return X


# revision 11
# speedup vs baseline: 2.2646x; 2.2646x over previous
"""TRN2 Bass kernel for the ESN (echo-state-network) recurrence:

    U   = inputs @ W_in + b_in                              # [B, T, N]
    x0  = 0.5 * tanh(U[:, 0])
    x_t = 0.5*x_{t-1} + 0.5*tanh(U[:, t] + x_{t-1} @ W_res + b_res)
    X   = stack([x0 ... x_{T-1}], 1)                        # [B, T, N]

Sharding: TIME-parallel with fading-memory warmup, two interleaved
chunks per core.  The ESN map is strongly contracting (leak 0.5,
spectral radius 0.9): a chunk's initial state is reconstructed by
running L warmup steps from x=0, with init error decaying below the
arithmetic noise floor well before L=12 (measured).

The sequence is split into 16 chunks of 32 output steps; core c runs
chunks 2c and 2c+1, interleaved step-by-step (A0 B0 A1 B1 ...).  The
two recurrences are independent, so while chunk A's post-matmul chain
(tanh -> blend -> transpose -> fp8 cast) completes, the PE runs chunk
B's full step: the sequential-dependence latency is fully hidden.
Same program on every core (SPMD); only input slices differ.  Chunk 0
of core 0 starts at t=0, where the program's step-0 formula is exactly
the reference's x0.

Per-step design ("x-stationary" matmul orientation, fp8 DoubleRow):
  - z[b, n] accumulates in PSUM [128, 1024] as lhsT.T @ rhs with the
    *state* stationary (xT8 fp8 tiles, 4 DoubleRow loads) and
    16*W_res as the fp8 moving operand: 8 DoubleRow matmuls per step
    stream the full N x N recurrent weight in ~1us.
  - The input projection + bias (scaled by 16 to match) is a K=65
    fp16 chunk accumulated into the same PSUM; tanh applies scale=1/16.
  - tanh on ScalarE (PSUM -> fp16), leak blend on VectorE, state
    transpose via the DMA Xbar engine (dma_start_transpose, off all
    compute engines, one [128,512]->[128,4,128] op per half), fp16->fp8
    cast on VectorE, x/2 bookkeeping on GpSimd.
  - Output written per step as [B, chunk, s, N] fp16; the host slices
    each chunk's valid 32-step window and concatenates.
"""

import sys

sys.path.insert(0, "/opt/trn_rl_repo")

from contextlib import ExitStack

import numpy as np

try:  # persistent jit cache so repeated runs skip long compiles
    import jax

    jax.config.update("jax_compilation_cache_dir", "/var/tmp/jax_comp_cache")
    jax.config.update("jax_persistent_cache_min_compile_time_secs", 0.0)
    jax.config.update("jax_persistent_cache_min_entry_size_bytes", 0)
except Exception:
    pass

import concourse.bass as bass
import concourse.tile as tile
from concourse import bacc, mybir
from concourse.bass_utils import run_bass_kernel_spmd

F32 = mybir.dt.float32
F16 = mybir.dt.float16
F8 = mybir.dt.float8e4

N_CORES = 8
B = 128  # full batch on every core
T = 512
D = 64
N = 1024
NC = 8  # N chunks of 128
P = 128
HN = 512  # half of N (one PSUM bank of fp32)
L = 12  # warmup steps per chunk
C2 = 32  # output steps per chunk
S2 = C2 + L  # program steps per chunk
NCHUNK = T // C2  # 16 chunks; 2 per core, interleaved
# State is stored DOUBLED (v = 2x) so the leak blend is a single op
# v_t = 0.5*v_{t-1} + tanh(.); the host halves the output.  The matmul
# needs x@W = v@(W/2), so W_res is scaled by WS/2 while the input
# projection uses WS; tanh rescales by 1/WS.
WS = 16.0  # fp8 weight scale (dodges e4m3 denormals; tanh rescales)
TANH = mybir.ActivationFunctionType.Tanh
ALU = mybir.AluOpType
DR = mybir.MatmulPerfMode.DoubleRow


def build_kernel():
    nc = bacc.Bacc(None, target_bir_lowering=False)
    # host-pretransposed inputs: inputs_t[d, b, j, s] = inputs[b, t0(j)+s, d]
    inputs = nc.dram_tensor("inputs_t", [D, B, 2, S2], F32, kind="ExternalInput")
    W_in = nc.dram_tensor("W_in", [D, N], F32, kind="ExternalInput")
    b_in = nc.dram_tensor("b_in", [N], F32, kind="ExternalInput")
    W_res = nc.dram_tensor("W_res", [N, N], F32, kind="ExternalInput")
    b_res = nc.dram_tensor("b_res", [N], F32, kind="ExternalInput")
    Xs = nc.dram_tensor("Xs", [B, 2, S2, N], F16, kind="ExternalOutput")

    K = D + 1  # input dims + ones row (bias via wi row 64)

    with tile.TileContext(nc) as tc, ExitStack() as ctx:
        consts = ctx.enter_context(tc.tile_pool(name="consts", bufs=1))
        stage = ctx.enter_context(tc.tile_pool(name="stage", bufs=1))
        state = ctx.enter_context(tc.tile_pool(name="state", bufs=2))
        zpool = ctx.enter_context(
            tc.tile_pool(name="zpool", bufs=2, space=bass.MemorySpace.PSUM)
        )

        # ---- W_res: wres8[p, k, n] = e4m3(16 * W_res[k*128+p, n]) ----
        wst = stage.tile([P, NC * N], F32, tag="stage", name="wst")
        nc.gpsimd.dma_start(
            out=wst.rearrange("p (k n) -> p k n", n=N),
            in_=W_res[:].rearrange("(k p) n -> p k n", p=P),
        )
        wres8 = consts.tile([P, NC, N], F8, tag="wres8")
        nc.vector.tensor_scalar_mul(wres8.rearrange("p k n -> p (k n)"), wst, WS / 2)

        # ---- wi tiles [65, N] (x16): rows 0..63 = 16*W_in; row 64 = 16*bias ----
        wi32 = consts.tile([K, N], F32, tag="wi32")
        wi032 = consts.tile([K, N], F32, tag="wi032")
        nc.gpsimd.dma_start(out=wi32[0:D], in_=W_in[:])
        nc.gpsimd.dma_start(out=wi032[0:D], in_=W_in[:])
        nc.gpsimd.dma_start(
            out=wi032[D : D + 1], in_=b_in[:].rearrange("(z n) -> z n", z=1)
        )
        bres_row = consts.tile([K, N], F32, tag="bres")
        nc.gpsimd.dma_start(
            out=bres_row[D : D + 1], in_=b_res[:].rearrange("(z n) -> z n", z=1)
        )
        nc.vector.tensor_tensor(
            out=wi32[D : D + 1],
            in0=wi032[D : D + 1],
            in1=bres_row[D : D + 1],
            op=ALU.add,
        )
        wi = consts.tile([K, N], F16, tag="wi")
        wi0 = consts.tile([K, N], F16, tag="wi0")
        nc.vector.tensor_scalar_mul(wi, wi32, WS)
        nc.vector.tensor_scalar_mul(wi0, wi032, WS)

        # ---- inputs: inp[d, b, j, s] fp16, row 64 = ones ----
        ist = stage.tile([D, B * 2 * S2], F32, tag="ist")
        nc.sync.dma_start(out=ist, in_=inputs[:].rearrange("d b j s -> d (b j s)"))
        inp = consts.tile([K, B, 2, S2], F16, tag="inp")
        nc.vector.tensor_copy(out=inp[0:D].rearrange("d b j s -> d (b j s)"), in_=ist)
        nc.vector.memset(inp[D : D + 1].rearrange("d b j s -> d (b j s)"), 1.0)

        xs_view = Xs[:]  # [B, 2, S2, N]
        hs0, hs1 = slice(0, HN), slice(HN, N)

        zt = {}

        def inp_mm(j, s):
            z = zpool.tile([P, N], F32, tag=f"z{j}", name=f"z{j}_{s}")
            zt[(j, s)] = z
            wi_use = wi0 if s == 0 else wi
            for h in range(2):
                nc.tensor.matmul(
                    z[:, h * HN : (h + 1) * HN],
                    inp[:, :, j, s],
                    wi_use[:, h * HN : (h + 1) * HN],
                    start=True,
                    stop=(s == 0),
                    skip_group_check=True,
                )

        # per-chunk rotating state (v = 2x)
        v_prev = [None, None]
        vT8_prev = [None, None]

        inp_mm(0, 0)
        inp_mm(1, 0)
        for i in range(2 * S2):
            j, s = i % 2, i // 2
            last = s == S2 - 1
            z = zt.pop((j, s))
            if s + 1 < S2:
                inp_mm(j, s + 1)
            if s > 0:
                vT8 = vT8_prev[j]
                for c in range(4):
                    for h in range(2):
                        nc.tensor.matmul(
                            z[:, h * HN : (h + 1) * HN],
                            vT8[:, 2 * c : 2 * c + 2, :],
                            wres8[:, 2 * c : 2 * c + 2, h * HN : (h + 1) * HN],
                            start=False,
                            stop=(c == 3),
                            perf_mode=DR,
                            skip_group_check=True,
                        )
            th = state.tile([P, N], F16, tag=f"th{j}", name=f"th{j}_{s}")
            vn = state.tile([P, N], F16, tag=f"vn{j}", name=f"vn{j}_{s}")
            for hs in (hs0, hs1):
                nc.scalar.activation(
                    out=th[:, hs], in_=z[:, hs], func=TANH, scale=1.0 / WS
                )
                if s == 0:
                    nc.vector.tensor_copy(out=vn[:, hs], in_=th[:, hs])
                else:
                    # v_t = 0.5*v_{t-1} + tanh(.)
                    nc.vector.scalar_tensor_tensor(
                        out=vn[:, hs], in0=v_prev[j][:, hs], scalar=0.5,
                        in1=th[:, hs], op0=ALU.mult, op1=ALU.add,
                    )
            if not last:
                # state transpose on the DMA Xbar (off compute engines),
                # then fp16 -> fp8 cast (ScalarE) for next step's stationary
                vT16 = state.tile([P, NC, P], F16, tag=f"vT16{j}", name=f"vT16{j}_{s}")
                vT8 = state.tile([P, NC, P], F8, tag=f"vT8{j}", name=f"vT8{j}_{s}")
                for h, hs in ((0, hs0), (1, hs1)):
                    nc.sync.dma_start_transpose(
                        out=vT16[:, 4 * h : 4 * h + 4, :], in_=vn[:, hs]
                    )
                    nc.scalar.copy(
                        out=vT8[:, 4 * h : 4 * h + 4, :].rearrange("p c b -> p (c b)"),
                        in_=vT16[:, 4 * h : 4 * h + 4, :].rearrange("p c b -> p (c b)"),
                    )
            else:
                vT8 = None
            nc.sync.dma_start(out=xs_view[:, j, s, :], in_=vn)
            v_prev[j], vT8_prev[j] = vn, vT8

    nc.compile()
    return nc


_NC_CACHE = {}


def _get_nc():
    if "nc" not in _NC_CACHE:
        _NC_CACHE["nc"] = build_kernel()
    return _NC_CACHE["nc"]


def _chunk_start(g):
    return 0 if g == 0 else C2 * g - L


def run_sharded(inputs, W_in, b_in, W_res, b_res, trace=False):
    """Run the SPMD kernel on 8 cores; returns (X_full, BassKernelResults)."""
    b_total, t_steps, _ = inputs.shape
    assert b_total == B and t_steps == T
    nc = _get_nc()
    shared = {
        "W_in": np.ascontiguousarray(W_in, np.float32),
        "b_in": np.ascontiguousarray(b_in, np.float32),
        "W_res": np.ascontiguousarray(W_res, np.float32),
        "b_res": np.ascontiguousarray(b_res, np.float32),
    }
    inputs = np.asarray(inputs, np.float32)
    in_maps = []
    for c in range(N_CORES):
        sl = np.empty((D, B, 2, S2), np.float32)
        for j in range(2):
            t0 = _chunk_start(2 * c + j)
            sl[:, :, j, :] = inputs[:, t0 : t0 + S2, :].transpose(2, 0, 1)
        in_maps.append({"inputs_t": np.ascontiguousarray(sl), **shared})
    res = run_bass_kernel_spmd(nc, in_maps, core_ids=list(range(N_CORES)), trace=trace)
    X = np.empty((B, T, N), np.float32)
    for c, r in enumerate(res.results):
        for j in range(2):
            g = 2 * c + j
            lo = 0 if g == 0 else L
            # stored state is v = 2x; halve during the fp32 upcast
            X[:, C2 * g : C2 * (g + 1), :] = r["Xs"][:, j, lo : lo + C2, :].astype(
                np.float32
            )
            X[:, C2 * g : C2 * (g + 1), :] *= 0.5
    return X, res


def kernel(**inputs):
    X, _ = run_sharded(
        inputs["inputs"],
        inputs["W_in"],
        inputs["b_in"],
        inputs["W_res"],
        inputs["b_res"],
    )
    return X
